# revision 7
# baseline (speedup 1.0000x reference)
"""Trainium2 Bass kernel for BeatPulseTransportCore.

Reference semantics (per batch row, R=160 bins, 3 channels):
  1. inject: h[:, :5, :] += (color*amount)[:,None,:] * w[None,:,None]; clip [0,1]
  2. advect (scatter-add with SCALAR offset): uniform 2-tap shift by
     k=floor(s) with weights p*(1-f), p*f; bins outside [0, R-1) dropped
  3. diffusion: [kd, 1-2kd, kd] stencil with zero boundary
  4. fade: last 8 bins scaled by ((R-1-idx)/8)^2

Because the advection offset is a scalar, steps 2+3 compose into a single
4-tap filter with CONSTANT coefficients along the bin axis:
  out[l] = sum_{d in taps} alpha_d * h~[l-d]
where h~ is h with invalid source bins dropped and zero-extension outside.
Instead of zero-padding SBUF blocks, each tap op is CLIPPED to the output
range whose source bins are valid; output bins no tap covers are memset
to zero.  This keeps each tile a single contiguous per-partition DMA.

This problem is memory-bound: per-core traffic is ~31.4 MB (f32 in+out)
against ~358 GB/s of HBM, i.e. ~88 us floor.  The kernel keeps every
compute engine's total busy time under that floor:
  - SP(sync) triggers input loads (HWDGE), one 15 KB contiguous
    descriptor per partition per tile
  - ACT does the per-q-block init tap (scaled copy)
  - DVE does inject + two accumulate taps per q-block (STT)
  - POOL does zero-memsets, one accumulate tap, fade, and triggers the
    output store via SWDGE so no compute engine queue blocks on it

Sharding: pure data parallel over batch across 8 cores (hint followed).
"""

import numpy as np

import concourse.bass as bass
import concourse.bacc as bacc
import concourse.mybir as mybir
from concourse import tile
from concourse.bass_utils import run_bass_kernel_spmd

R = 160
C = 3
RC = R * C
FADE_W = 8
N_CORES = 8
B_FULL = 65536

f32 = np.float32
FP = mybir.dt.float32


def host_constants(
    offset_per_frame_60hz,
    persistence_per_frame_60hz,
    diffusion01,
    dt_seconds,
    amount01,
    spread01,
):
    offset, persistence = offset_per_frame_60hz, persistence_per_frame_60hz
    """Replicate the reference's f32 scalar math on host; returns everything
    the device program needs."""
    offset = f32(offset)
    persistence = f32(persistence)
    diffusion01 = f32(diffusion01)
    dt_seconds = f32(dt_seconds)
    amount01 = f32(amount01)
    spread01 = f32(spread01)

    dt = np.clip(dt_seconds, f32(0.0), f32(0.05)).astype(f32)
    dt_scale = f32(dt * f32(60.0))
    s = f32(offset * dt_scale)
    p = f32(persistence**dt_scale)

    amount = np.clip(amount01, f32(0.0), f32(1.0)).astype(f32)
    spread = np.clip(spread01, f32(0.0), f32(1.0)).astype(f32)
    tight = f32(f32(1.0) - spread)
    w5 = np.array(
        [
            f32(f32(0.5) + f32(0.4) * tight),
            f32(f32(0.2) * spread + f32(0.05)),
            f32(f32(0.12) * spread),
            f32(f32(0.06) * spread),
            f32(f32(0.02) * spread),
        ],
        dtype=f32,
    )

    # advect geometry, exactly as the reference computes it in f32
    i_idx = np.arange(R, dtype=f32)
    new_pos = (i_idx + s).astype(f32)
    valid = (new_pos >= f32(0.0)) & (new_pos < f32(R - 1))
    left = np.clip(np.floor(new_pos).astype(np.int32), 0, R - 2)
    frac = (new_pos - left.astype(f32)).astype(f32)

    kd = f32(f32(0.15) * diffusion01)
    cc = f32(f32(1.0) - f32(2.0) * kd)

    fade = np.ones(R, dtype=f32)
    idx = np.arange(R)
    t = ((R - 1 - idx).astype(f32) / f32(FADE_W)).astype(f32)
    fade = np.where(idx >= R - FADE_W, (t * t).astype(f32), fade).astype(f32)

    out = {
        "valid": valid,
        "left": left,
        "frac": frac,
        "p": p,
        "kd": kd,
        "cc": cc,
        "fade": fade,
        "w5": w5,
        "amount": amount,
    }

    if not valid.any():
        out.update(k=0, f=f32(0.0), i_min=0, i_max=-1, deviants=[], alphas={})
        return out

    iv = np.nonzero(valid)[0]
    i_min, i_max = int(iv[0]), int(iv[-1])
    shifts = left - np.arange(R, dtype=np.int32)
    vals, counts = np.unique(shifts[valid], return_counts=True)
    k = int(vals[np.argmax(counts)])
    nondev = iv[shifts[iv] == k]
    f = f32(frac[nondev[len(nondev) // 2]])

    wl = f32(f32(f32(1.0) - f) * p)
    wr = f32(f * p)
    alphas = {
        k - 1: float(kd * wl),
        k: float(cc * wl + kd * wr),
        k + 1: float(cc * wr + kd * wl),
        k + 2: float(kd * wr),
    }

    # rows whose f32-rounded floor lands on a different integer shift;
    # corrected with a few tiny extra instructions (measure-zero case).
    deviants = []
    for i in iv[shifts[iv] != k]:
        i = int(i)
        wl_i = f32(f32(f32(1.0) - frac[i]) * p)
        wr_i = f32(frac[i] * p)
        # per-output-column coefficient delta: true minus what the uniform
        # pass already applied for source column i
        true_c = {}
        for j, wgt in ((int(left[i]), wl_i), (int(left[i]) + 1, wr_i)):
            for l, dw in ((j - 1, kd), (j, cc), (j + 1, kd)):
                if 0 <= l < R:
                    true_c[l] = true_c.get(l, 0.0) + float(wgt) * float(dw)
        assumed_c = {}
        for d, a in alphas.items():
            l = i + d
            if 0 <= l < R:
                assumed_c[l] = a
        cols = sorted(set(true_c) | set(assumed_c))
        fix = []
        for l in cols:
            delta = (true_c.get(l, 0.0) - assumed_c.get(l, 0.0)) * float(fade[l])
            if delta != 0.0:
                fix.append((l, delta))
        if fix:
            deviants.append((i, fix))

    out.update(k=k, f=f, i_min=i_min, i_max=i_max, deviants=deviants, alphas=alphas)
    return out


def build_program(
    n_rows,
    consts,
    Q=8,
    in_bufs=5,
    out_bufs=3,
    tail_eng="pool",
    init_eng="act",
    fade_eng="pool",
    store_eng="pool",
    qgrp=1,
):
    """Build the single-core Bass/Tile program for a batch shard of n_rows.

    Layout: partition p of tile t holds batch rows r0 + p*Q + (0..Q-1), so
    every tile's load and store is one contiguous 15360-byte descriptor per
    partition.

    Per q-block the 4-tap filter is: init tap (scaled copy on ACT), two
    accumulate taps (STT on DVE; the Pool engine does not implement STT),
    and a tail tap computed as an ACT prescale into a temp followed by a
    plain tensor_tensor add on `tail_eng`.
    init_eng: engine for the init tap ("act" scaled copy, or "dve" TS mul).
    store_eng: which engine triggers the output store DMA ("pool" = SWDGE
      from the gpsimd queue so ACT/SP never block on compute; "act"/"sync"
      = HWDGE).
    """
    RT = 128 * Q  # rows per tile
    assert n_rows % RT == 0
    n_tiles = n_rows // RT

    alphas = consts["alphas"]
    taps = [(d, alphas[d]) for d in sorted(alphas) if alphas[d] != 0.0]
    i_min, i_max = consts["i_min"], consts["i_max"]
    fade = consts["fade"]
    w5 = consts["w5"]
    amount = consts["amount"]
    valid = consts["valid"]

    # per-tap output coverage in bins: sources clipped to valid loaded bins
    cov = {}
    for d, a in taps:
        lo = max(0, i_min + d)
        hi = min(R - 1, i_max + d)
        if lo <= hi:
            cov[d] = (lo, hi)
    taps = [(d, a) for d, a in taps if d in cov]
    have_work = len(taps) > 0

    if have_work:
        # init tap: widest coverage, ties broken by |alpha|
        init_d = max(taps, key=lambda da: (cov[da[0]][1] - cov[da[0]][0], abs(da[1])))[0]
        init_a = dict(taps)[init_d]
        rest = [(d, a) for d, a in taps if d != init_d]
        # tail tap: prescaled on ACT, combined with a plain add (only when
        # there are enough taps to make offloading worthwhile)
        if len(rest) >= 2:
            chain_taps, tail_tap = rest[:-1], rest[-1]
        else:
            chain_taps, tail_tap = rest, None
        lo0, hi0 = cov[init_d]
        zero_regions = []
        if lo0 > 0:
            zero_regions.append((0, lo0))
        if hi0 < R - 1:
            zero_regions.append((hi0 + 1, R))

    fade_cols = R - FADE_W  # first faded bin
    fade_vec = np.tile(fade[fade_cols:].repeat(C), Q).astype(f32)  # [Q*24]
    fade_const_np = np.broadcast_to(fade_vec, (128, Q * FADE_W * C)).copy()

    # injection weights, [p, 3j+c] = amount*w[j] (masked to advect-kept bins)
    wrow_vec = np.zeros(15, dtype=f32)
    for j in range(5):
        if valid[j]:
            wrow_vec[3 * j : 3 * j + 3] = f32(amount * w5[j])
    wrow_const_np = np.broadcast_to(wrow_vec, (128, 15)).copy()

    nc = bacc.Bacc(None)
    hist = nc.dram_tensor("history", [n_rows, R, C], FP, kind="ExternalInput")
    color = nc.dram_tensor("color_rgb", [n_rows, C], FP, kind="ExternalInput")
    fade_dram = nc.dram_tensor("fade_const", [128, Q * FADE_W * C], FP, kind="ExternalInput")
    wrow_dram = nc.dram_tensor("wrow_const", [128, 15], FP, kind="ExternalInput")
    out = nc.dram_tensor("out", [n_rows, R, C], FP, kind="ExternalOutput")

    mult = mybir.AluOpType.mult
    add = mybir.AluOpType.add
    amin = mybir.AluOpType.min
    amax = mybir.AluOpType.max

    engs = {}

    with tile.TileContext(nc) as tc:
        engs = {"dve": nc.vector, "pool": nc.gpsimd}
        with (
            tc.tile_pool(name="const", bufs=1) as cpool,
            tc.tile_pool(name="data", bufs=in_bufs) as dpool,
            tc.tile_pool(name="outp", bufs=out_bufs) as opool,
            tc.tile_pool(name="tmp", bufs=2) as tpool,
        ):
            fade_t = cpool.tile([128, Q * FADE_W * C], FP)
            nc.sync.dma_start(fade_t[:], fade_dram[:])
            wrow_t = cpool.tile([128, 15], FP)
            nc.sync.dma_start(wrow_t[:], wrow_dram[:])

            for t in range(n_tiles):
                r0 = t * RT
                h_t = dpool.tile([128, Q * RC], FP)
                col_t = dpool.tile([128, Q * C], FP)
                o_t = opool.tile([128, Q * RC], FP)

                h3 = h_t.rearrange("p (q f) -> p q f", f=RC)
                o3 = o_t.rearrange("p (q f) -> p q f", f=RC)

                # one contiguous 15360B per-partition descriptor per tile
                nc.sync.dma_start(
                    h_t[:], hist[r0 : r0 + RT].rearrange("(p q) r c -> p (q r c)", q=Q)
                )
                nc.sync.dma_start(
                    col_t[:], color[r0 : r0 + RT].rearrange("(p q) c -> p (q c)", q=Q)
                )

                if have_work:
                    # inject into bins 0..4 (advect-dropped bins masked in
                    # wrow): h[:, :, :5, :] += color[:, None, :] * wrow; clamp
                    inj_t = dpool.tile([128, Q * 15], FP)
                    inj4 = inj_t.rearrange("p (q j c) -> p q j c", j=5, c=C)
                    colb = (
                        col_t.rearrange("p (q c) -> p q c", c=C)
                        .unsqueeze(2)
                        .broadcast_to((128, Q, 5, C))
                    )
                    wrowb = (
                        wrow_t.rearrange("p (j c) -> p j c", c=C)
                        .unsqueeze(1)
                        .broadcast_to((128, Q, 5, C))
                    )
                    nc.vector.tensor_tensor(inj4, colb, wrowb, mult)
                    hinj = h3[:, :, 0:15]
                    nc.vector.tensor_tensor(
                        hinj, hinj, inj_t.rearrange("p (q f) -> p q f", f=15), add
                    )
                    nc.vector.tensor_scalar(hinj, hinj, 1.0, 0.0, amin, amax)

                    # zero the output bins the init tap does not cover
                    for zlo, zhi in zero_regions:
                        nc.gpsimd.memset(o3[:, :, 3 * zlo : 3 * zhi], 0.0)

                    # per-q-block tap chains: init (scaled copy on ACT),
                    # STT accumulates on DVE, tail tap prescaled on ACT and
                    # added with a plain tensor_tensor on tail_eng
                    if tail_tap is not None:
                        tlo, thi = cov[tail_tap[0]]
                        tlen = 3 * (thi + 1 - tlo)
                        tmp_t = tpool.tile([128, Q * tlen], FP)
                        tmp3 = tmp_t.rearrange("p (q f) -> p q f", f=tlen)
                    for q0 in range(0, Q, qgrp):
                        qs = slice(q0, q0 + qgrp)
                        lo, hi = cov[init_d]
                        osl = o3[:, qs, 3 * lo : 3 * (hi + 1)]
                        hsl = h3[:, qs, 3 * (lo - init_d) : 3 * (hi + 1 - init_d)]
                        if init_eng == "act":
                            nc.scalar.mul(osl, hsl, float(init_a))
                        else:
                            nc.vector.tensor_scalar(osl, hsl, float(init_a), 0.0, mult, add)
                        for d, a in chain_taps:
                            lo, hi = cov[d]
                            osl = o3[:, qs, 3 * lo : 3 * (hi + 1)]
                            hsl = h3[:, qs, 3 * (lo - d) : 3 * (hi + 1 - d)]
                            nc.vector.scalar_tensor_tensor(
                                osl, hsl, float(a), osl, mult, add
                            )
                        if tail_tap is not None:
                            d, a = tail_tap
                            tsl = tmp3[:, qs, :]
                            hsl = h3[:, qs, 3 * (tlo - d) : 3 * (thi + 1 - d)]
                            nc.scalar.mul(tsl, hsl, float(a))
                            osl = o3[:, qs, 3 * tlo : 3 * (thi + 1)]
                            engs[tail_eng].tensor_tensor(osl, osl, tsl, add)

                    # fade on the last 8 bins
                    engs[fade_eng].tensor_tensor(
                        o3[:, :, fade_cols * C :],
                        o3[:, :, fade_cols * C :],
                        fade_t.rearrange("p (q f) -> p q f", f=FADE_W * C),
                        mult,
                    )

                    # sparse fixups for f32 rounding deviants (rarely present)
                    for i, fix in consts["deviants"]:
                        hcol = h3[:, :, 3 * i : 3 * (i + 1)]
                        for l, delta in fix:
                            ocol = o3[:, :, 3 * l : 3 * (l + 1)]
                            nc.vector.scalar_tensor_tensor(
                                ocol, hcol, float(delta), ocol, mult, add
                            )
                else:
                    nc.gpsimd.memset(o_t[:], 0.0)

                dst = out[r0 : r0 + RT].rearrange("(p q) r c -> p (q r c)", q=Q)
                if store_eng == "pool":
                    nc.gpsimd.dma_start(dst, o_t[:])
                elif store_eng == "act":
                    nc.scalar.dma_start(dst, o_t[:])
                else:
                    nc.sync.dma_start(dst, o_t[:])

    nc.compile()
    const_inputs = {
        "fade_const": fade_const_np,
        "wrow_const": wrow_const_np,
    }
    return nc, const_inputs


def kernel(
    history,
    color_rgb,
    offset_per_frame_60hz,
    persistence_per_frame_60hz,
    diffusion01,
    dt_seconds,
    amount01,
    spread01,
):
    history = np.asarray(history, dtype=np.float32)
    color_rgb = np.asarray(color_rgb, dtype=np.float32)
    B = history.shape[0]
    assert B % N_CORES == 0
    shard = B // N_CORES

    consts = host_constants(
        offset_per_frame_60hz,
        persistence_per_frame_60hz,
        diffusion01,
        dt_seconds,
        amount01,
        spread01,
    )

    nc, const_inputs = build_program(shard, consts, **BUILD_OVERRIDES)

    in_maps = []
    for cid in range(N_CORES):
        sl = slice(cid * shard, (cid + 1) * shard)
        in_maps.append(
            {"history": history[sl], "color_rgb": color_rgb[sl], **const_inputs}
        )

    res = run_bass_kernel_spmd(nc, in_maps, core_ids=list(range(N_CORES)), **RUN_KWARGS)
    global LAST_RESULT
    LAST_RESULT = res
    return np.concatenate([res.results[i]["out"] for i in range(N_CORES)], axis=0)


# test-harness hooks (unused when graded: defaults are plain execution)
RUN_KWARGS: dict = {}
BUILD_OVERRIDES: dict = {}
LAST_RESULT = None


# revision 12
# speedup vs baseline: 1.2782x; 1.2782x over previous
"""Trainium2 Bass kernel for BeatPulseTransportCore.

Reference semantics (per batch row, R=160 bins, 3 channels):
  1. inject: h[:, :5, :] += (color*amount)[:,None,:] * w[None,:,None]; clip [0,1]
  2. advect (scatter-add with SCALAR offset): uniform 2-tap shift by
     k=floor(s) with weights p*(1-f), p*f; bins outside [0, R-1) dropped
  3. diffusion: [kd, 1-2kd, kd] stencil with zero boundary
  4. fade: last 8 bins scaled by ((R-1-idx)/8)^2

Because the advection offset is a scalar, steps 2+3 compose into a single
4-tap filter with CONSTANT coefficients along the bin axis:
  out[l] = sum_{d in taps} alpha_d * h~[l-d]
where h~ is h with invalid source bins zeroed and zero-extension outside.

This problem is memory-bound: per-core traffic is ~31.4 MB (f32 in+out)
against ~358 GB/s of HBM, i.e. an ~88 us floor.  The elementwise engines
(DVE/ACT/POOL) cannot absorb 4 full passes over the data under that floor,
so the filter runs on the otherwise-idle TensorEngine in bf16:

  - history is loaded with an f32->bf16 cast during the SWDGE DMA
    (HBM reads stay f32-sized; tolerance is 2e-2 so bf16 is plenty),
    into per-q-block zero-padded slots so every tap is a full-width
    shifted window
  - inject runs in bf16 on DVE (3 small ops on 15 cols/q-block)
  - each tap is ONE bf16 matmul per half-tile against a scaled-identity
    stationary matrix, accumulating in f32 PSUM across 4 banks
    (multi-q moving pattern amortizes the per-matmul LDWEIGHTS)
  - ACT copies PSUM -> f32 SBUF; DVE applies the per-column fade;
    stores are plain f32 HWDGE on the otherwise-empty SP queue

Every tile's load and store is one contiguous per-partition descriptor
(partition p of tile t holds batch rows r0 + p*Q + (0..Q-1)).

Sharding: pure data parallel over batch across 8 cores (hint followed).
"""

import numpy as np
import ml_dtypes

import concourse.bass as bass
import concourse.bacc as bacc
import concourse.mybir as mybir
from concourse import tile
from concourse.bass_utils import run_bass_kernel_spmd

R = 160
C = 3
RC = R * C
FADE_W = 8
N_CORES = 8
B_FULL = 65536

f32 = np.float32
bf16 = ml_dtypes.bfloat16
FP = mybir.dt.float32
BF = mybir.dt.bfloat16


def host_constants(
    offset_per_frame_60hz,
    persistence_per_frame_60hz,
    diffusion01,
    dt_seconds,
    amount01,
    spread01,
):
    offset, persistence = offset_per_frame_60hz, persistence_per_frame_60hz
    """Replicate the reference's f32 scalar math on host; returns everything
    the device program needs."""
    offset = f32(offset)
    persistence = f32(persistence)
    diffusion01 = f32(diffusion01)
    dt_seconds = f32(dt_seconds)
    amount01 = f32(amount01)
    spread01 = f32(spread01)

    dt = np.clip(dt_seconds, f32(0.0), f32(0.05)).astype(f32)
    dt_scale = f32(dt * f32(60.0))
    s = f32(offset * dt_scale)
    p = f32(persistence**dt_scale)

    amount = np.clip(amount01, f32(0.0), f32(1.0)).astype(f32)
    spread = np.clip(spread01, f32(0.0), f32(1.0)).astype(f32)
    tight = f32(f32(1.0) - spread)
    w5 = np.array(
        [
            f32(f32(0.5) + f32(0.4) * tight),
            f32(f32(0.2) * spread + f32(0.05)),
            f32(f32(0.12) * spread),
            f32(f32(0.06) * spread),
            f32(f32(0.02) * spread),
        ],
        dtype=f32,
    )

    # advect geometry, exactly as the reference computes it in f32
    i_idx = np.arange(R, dtype=f32)
    new_pos = (i_idx + s).astype(f32)
    valid = (new_pos >= f32(0.0)) & (new_pos < f32(R - 1))
    left = np.clip(np.floor(new_pos).astype(np.int32), 0, R - 2)
    frac = (new_pos - left.astype(f32)).astype(f32)

    kd = f32(f32(0.15) * diffusion01)
    cc = f32(f32(1.0) - f32(2.0) * kd)

    fade = np.ones(R, dtype=f32)
    idx = np.arange(R)
    t = ((R - 1 - idx).astype(f32) / f32(FADE_W)).astype(f32)
    fade = np.where(idx >= R - FADE_W, (t * t).astype(f32), fade).astype(f32)

    out = {
        "valid": valid,
        "left": left,
        "frac": frac,
        "p": p,
        "kd": kd,
        "cc": cc,
        "fade": fade,
        "w5": w5,
        "amount": amount,
    }

    if not valid.any():
        out.update(k=0, f=f32(0.0), i_min=0, i_max=-1, deviants=[], alphas={})
        return out

    iv = np.nonzero(valid)[0]
    i_min, i_max = int(iv[0]), int(iv[-1])
    shifts = left - np.arange(R, dtype=np.int32)
    vals, counts = np.unique(shifts[valid], return_counts=True)
    k = int(vals[np.argmax(counts)])
    nondev = iv[shifts[iv] == k]
    f = f32(frac[nondev[len(nondev) // 2]])

    wl = f32(f32(f32(1.0) - f) * p)
    wr = f32(f * p)
    alphas = {
        k - 1: float(kd * wl),
        k: float(cc * wl + kd * wr),
        k + 1: float(cc * wr + kd * wl),
        k + 2: float(kd * wr),
    }

    # rows whose f32-rounded floor lands on a different integer shift;
    # corrected with a few tiny extra instructions (measure-zero case).
    deviants = []
    for i in iv[shifts[iv] != k]:
        i = int(i)
        wl_i = f32(f32(f32(1.0) - frac[i]) * p)
        wr_i = f32(frac[i] * p)
        # per-output-column coefficient delta: true minus what the uniform
        # pass already applied for source column i
        true_c = {}
        for j, wgt in ((int(left[i]), wl_i), (int(left[i]) + 1, wr_i)):
            for l, dw in ((j - 1, kd), (j, cc), (j + 1, kd)):
                if 0 <= l < R:
                    true_c[l] = true_c.get(l, 0.0) + float(wgt) * float(dw)
        assumed_c = {}
        for d, a in alphas.items():
            l = i + d
            if 0 <= l < R:
                assumed_c[l] = a
        cols = sorted(set(true_c) | set(assumed_c))
        fix = []
        for l in cols:
            delta = (true_c.get(l, 0.0) - assumed_c.get(l, 0.0)) * float(fade[l])
            if delta != 0.0:
                fix.append((l, delta))
        if fix:
            deviants.append((i, fix))

    out.update(k=k, f=f, i_min=i_min, i_max=i_max, deviants=deviants, alphas=alphas)
    return out


def build_program(
    n_rows,
    consts,
    Q=8,
    in_bufs=6,
    out_bufs=3,
    QG=1,
    fade_eng="dve",
    store_eng="sync",
):
    """Build the single-core Bass/Tile program for a batch shard of n_rows.

    QG: q-blocks per PSUM accumulation group (QG*512 f32 <= 8 PSUM banks
        per group; QG=4 ping-pongs two groups across the 8 banks).
    """
    RT = 128 * Q  # rows per tile
    assert n_rows % RT == 0
    assert Q % QG == 0
    n_tiles = n_rows // RT

    alphas = consts["alphas"]
    taps = [(d, alphas[d]) for d in sorted(alphas) if alphas[d] != 0.0]
    i_min, i_max = consts["i_min"], consts["i_max"]
    fade = consts["fade"]
    w5 = consts["w5"]
    amount = consts["amount"]
    valid = consts["valid"]
    # per-tap output coverage in bins: sources clipped to valid loaded bins
    cov = {}
    for d, a in taps:
        lo = max(0, i_min + d)
        hi = min(R - 1, i_max + d)
        if lo <= hi:
            cov[d] = (lo, hi)
    taps = [(d, a) for d, a in taps if d in cov]
    have_work = len(taps) > 0

    if have_work:
        # start tap (PSUM start=True resets its columns): widest coverage;
        # the uncovered edge columns are zeroed with a small PSUM memset
        start_i = max(
            range(len(taps)),
            key=lambda i: (cov[taps[i][0]][1] - cov[taps[i][0]][0], abs(taps[i][1])),
        )
        taps = [taps[start_i]] + taps[:start_i] + taps[start_i + 1 :]
        lo0, hi0 = cov[taps[0][0]]
        zero_regions = []
        if lo0 > 0:
            zero_regions.append((0, 3 * lo0))
        if hi0 < R - 1:
            zero_regions.append((3 * (hi0 + 1), RC))

    fade_cols = R - FADE_W  # first faded bin
    fade_vec = np.tile(fade[fade_cols:].repeat(C), Q).astype(f32)  # [Q*24]
    fade_const_np = np.broadcast_to(fade_vec, (128, Q * FADE_W * C)).copy()

    # injection weights, [p, 3j+c] = amount*w[j] (masked to advect-kept bins)
    wrow_vec = np.zeros(15, dtype=f32)
    for j in range(5):
        if valid[j]:
            wrow_vec[3 * j : 3 * j + 3] = f32(amount * w5[j])
    wrow_const_np = np.broadcast_to(wrow_vec, (128, 15)).copy().astype(bf16)

    # scaled identities for the PE taps: eye[p, di*128+m] = alpha_d * (p==m)
    n_taps = max(len(taps), 1)
    eye_const_np = np.zeros((128, n_taps * 128), dtype=bf16)
    for di, (d, a) in enumerate(taps):
        eye_const_np[np.arange(128), di * 128 + np.arange(128)] = bf16(a)

    nc = bacc.Bacc(None)
    hist = nc.dram_tensor("history", [n_rows, R, C], FP, kind="ExternalInput")
    color = nc.dram_tensor("color_rgb", [n_rows, C], FP, kind="ExternalInput")
    fade_dram = nc.dram_tensor("fade_const", [128, Q * FADE_W * C], FP, kind="ExternalInput")
    wrow_dram = nc.dram_tensor("wrow_const", [128, 15], BF, kind="ExternalInput")
    eye_dram = nc.dram_tensor("eye_const", [128, n_taps * 128], BF, kind="ExternalInput")
    out = nc.dram_tensor("out", [n_rows, R, C], FP, kind="ExternalOutput")

    mult = mybir.AluOpType.mult
    add = mybir.AluOpType.add
    amin = mybir.AluOpType.min
    amax = mybir.AluOpType.max

    with tile.TileContext(nc) as tc:
        with (
            tc.tile_pool(name="const", bufs=1) as cpool,
            tc.tile_pool(name="data", bufs=in_bufs) as dpool,
            tc.tile_pool(name="outp", bufs=out_bufs) as opool,
            tc.tile_pool(name="ps", bufs=2, space="PSUM") as pspool,
        ):
            fade_t = cpool.tile([128, Q * FADE_W * C], FP)
            nc.sync.dma_start(fade_t[:], fade_dram[:])
            wrow_t = cpool.tile([128, 15], BF)
            nc.sync.dma_start(wrow_t[:], wrow_dram[:])
            eye_t = cpool.tile([128, n_taps * 128], BF)
            nc.sync.dma_start(eye_t[:], eye_dram[:])

            for t in range(n_tiles):
                r0 = t * RT
                h_t = dpool.tile([128, Q * RC], BF)
                col_t = dpool.tile([128, Q * C], BF)
                o_t = opool.tile([128, Q * RC], FP)

                h3 = h_t.rearrange("p (q f) -> p q f", f=RC)
                o3 = o_t.rearrange("p (q f) -> p q f", f=RC)

                if have_work:
                    # f32 -> bf16 cast during the SWDGE load; one descriptor
                    # per partition (contiguous rows p*Q..p*Q+Q-1, full width
                    # incl. invalid bins, which the clipped taps never read)
                    nc.gpsimd.dma_start(
                        h_t[:],
                        hist[r0 : r0 + RT].rearrange("(p q) r c -> p (q r c)", q=Q),
                    )
                    nc.gpsimd.dma_start(
                        col_t[:],
                        color[r0 : r0 + RT].rearrange("(p q) c -> p (q c)", q=Q),
                    )

                    # inject into bins 0..4 (advect-dropped bins masked in
                    # wrow): h[:, :5, :] += color[:, None, :] * wrow; clamp
                    inj_t = dpool.tile([128, Q * 15], BF)
                    inj4 = inj_t.rearrange("p (q j c) -> p q j c", j=5, c=C)
                    colb = (
                        col_t.rearrange("p (q c) -> p q c", c=C)
                        .unsqueeze(2)
                        .broadcast_to((128, Q, 5, C))
                    )
                    wrowb = (
                        wrow_t.rearrange("p (j c) -> p j c", c=C)
                        .unsqueeze(1)
                        .broadcast_to((128, Q, 5, C))
                    )
                    nc.vector.tensor_tensor(inj4, colb, wrowb, mult)
                    hinj = h3[:, :, 0:15]
                    nc.vector.tensor_tensor(
                        hinj, hinj, inj_t.rearrange("p (q f) -> p q f", f=15), add
                    )
                    nc.vector.tensor_scalar(hinj, hinj, 1.0, 0.0, amin, amax)

                    # 4-tap filter on TensorE: per QG-group, one bf16 matmul
                    # per tap with a multi-q moving pattern and clipped
                    # ranges, accumulated in f32 PSUM (QG banks per group);
                    # edge columns the start tap doesn't cover are zeroed
                    for g0 in range(0, Q, QG):
                        ps_t = pspool.tile([128, QG * 512], FP)
                        ps3 = ps_t.rearrange("p (g f) -> p g f", f=512)
                        for zlo, zhi in zero_regions:
                            nc.vector.memset(ps3[:, :, zlo:zhi], 0.0)
                        for di, (d, a) in enumerate(taps):
                            lo, hi = cov[d]
                            nc.tensor.matmul(
                                ps3[:, :, 3 * lo : 3 * (hi + 1)],
                                eye_t[:, di * 128 : (di + 1) * 128],
                                h3[:, g0 : g0 + QG, 3 * (lo - d) : 3 * (hi + 1 - d)],
                                start=(di == 0),
                                stop=(di == len(taps) - 1),
                                skip_group_check=True,
                            )
                        nc.scalar.copy(o3[:, g0 : g0 + QG, :], ps3[:, :, 0:RC])

                    # fade on the last 8 bins
                    feng = nc.vector if fade_eng == "dve" else nc.gpsimd
                    feng.tensor_tensor(
                        o3[:, :, fade_cols * C :],
                        o3[:, :, fade_cols * C :],
                        fade_t.rearrange("p (q f) -> p q f", f=FADE_W * C),
                        mult,
                    )

                    # sparse fixups for f32 rounding deviants (rarely present)
                    for i, fix in consts["deviants"]:
                        hcol = h3[:, :, 3 * i : 3 * (i + 1)]
                        for l, delta in fix:
                            ocol = o3[:, :, 3 * l : 3 * (l + 1)]
                            nc.vector.scalar_tensor_tensor(
                                ocol, hcol, float(delta), ocol, mult, add
                            )
                else:
                    nc.gpsimd.memset(o_t[:], 0.0)

                dst = out[r0 : r0 + RT].rearrange("(p q) r c -> p (q r c)", q=Q)
                if store_eng == "sync":
                    nc.sync.dma_start(dst, o_t[:])
                elif store_eng == "act":
                    nc.scalar.dma_start(dst, o_t[:])
                else:
                    nc.gpsimd.dma_start(dst, o_t[:])

    nc.compile()
    const_inputs = {
        "fade_const": fade_const_np,
        "wrow_const": wrow_const_np,
        "eye_const": eye_const_np,
    }
    return nc, const_inputs


def kernel(
    history,
    color_rgb,
    offset_per_frame_60hz,
    persistence_per_frame_60hz,
    diffusion01,
    dt_seconds,
    amount01,
    spread01,
):
    history = np.asarray(history, dtype=np.float32)
    color_rgb = np.asarray(color_rgb, dtype=np.float32)
    B = history.shape[0]
    assert B % N_CORES == 0
    shard = B // N_CORES

    consts = host_constants(
        offset_per_frame_60hz,
        persistence_per_frame_60hz,
        diffusion01,
        dt_seconds,
        amount01,
        spread01,
    )

    nc, const_inputs = build_program(shard, consts, **BUILD_OVERRIDES)

    in_maps = []
    for cid in range(N_CORES):
        sl = slice(cid * shard, (cid + 1) * shard)
        in_maps.append(
            {"history": history[sl], "color_rgb": color_rgb[sl], **const_inputs}
        )

    res = run_bass_kernel_spmd(nc, in_maps, core_ids=list(range(N_CORES)), **RUN_KWARGS)
    global LAST_RESULT
    LAST_RESULT = res
    return np.concatenate([res.results[i]["out"] for i in range(N_CORES)], axis=0)


# test-harness hooks (unused when graded: defaults are plain execution)
RUN_KWARGS: dict = {}
BUILD_OVERRIDES: dict = {}
LAST_RESULT = None
